# revision 26
# baseline (speedup 1.0000x reference)
"""MultiHeadSelfAttention Trainium2 Bass kernel.

Shapes (hardcoded): B=8, N=2048, E=512, H=8 heads, D=64 head dim.
Sharding: data-parallel over batch -> one batch item per NeuronCore (8 cores),
no collectives needed.

Per-core pipeline (bf16 compute, fp32 accumulate). The attention phase is
bound by the Activation engine (the only engine with Exp): 256 exps of
[128,1024] at ~1.09us each. Everything else is arranged so no other engine
ever blocks that stream:

  loads: DMA order Wv,Wk,Wq | V | K0 Q0 K1..K3 | Q1..Q3 | Wo. Per-512-row
         group: f32 load, bf16 cast (DVE/ACT/GPSIMD round-robin),
         PE-transpose into X^T, project (v_aug / kT / qT) as inputs land.
         The first attention chunk is woven between K-group arrivals so the
         Exp stream starts as early as possible.
  attention (h outer, 512-query chunk inner, key-tile pair per step):
         S^T [128 keys, 2x512] in PSUM = k^T . q^T; Exp on ScalarE (1/tau
         folded into scale) -> P^T bf16; O^T [65,512] += v_aug^T . P^T over
         16 key tiles; row 64 = softmax denominator (ones column of v_aug).
         Remaining qk projections, Wo prep, output projection and stores are
         interleaved as PE filler.
  norm (per chunk, eager, no PE): O row 64 -> DRAM, broadcast-DMA back
         across 64 partitions, reciprocal_approx_fast, multiply into oTn.
  tail: yT = Wo @ oTn + bo per column chunk as h7 filler; PE-transpose back
         to [2048,512]; cast fp32; DMA out.

The attention mask is all ones per the problem spec; validated host-side.
"""

import sys

for _p in ("/opt/trn_rl_repo",):
    if _p not in sys.path:
        sys.path.insert(0, _p)

import numpy as np
from collections import deque
from contextlib import ExitStack

import concourse.bass as bass
import concourse.bacc as bacc
import concourse.mybir as mybir
import concourse.tile as tile
from concourse.masks import make_identity

B, N, E = 8, 2048, 512
H, D = 8, 64
P = 128          # partitions
ET = E // P      # 4 e-tiles
NT = N // P      # 16 n-tiles
QC = 512         # q chunk in attention
NQC = N // QC    # 4
KTP = NT // 2    # 8 key-tile pairs
NG = NT // ET    # 4 row-groups of 512 per input
HV = 65          # head dim + ones column
FP32 = mybir.dt.float32
BF16 = mybir.dt.bfloat16
NCORES = 8

AF = mybir.ActivationFunctionType
ALU = mybir.AluOpType


def _build(inv_tau: float) -> bass.Bass:
    nc = bacc.Bacc(trn_type="TRN2")

    dQ = nc.dram_tensor("Q", [N, E], FP32, kind="ExternalInput")
    dK = nc.dram_tensor("K", [N, E], FP32, kind="ExternalInput")
    dV = nc.dram_tensor("V", [N, E], FP32, kind="ExternalInput")
    dWq = nc.dram_tensor("Wq", [E, E], FP32, kind="ExternalInput")
    dWk = nc.dram_tensor("Wk", [E, E], FP32, kind="ExternalInput")
    dWv = nc.dram_tensor("Wv", [E, E], FP32, kind="ExternalInput")
    dWo = nc.dram_tensor("Wo", [E, E], FP32, kind="ExternalInput")
    dbo = nc.dram_tensor("bo", [E], FP32, kind="ExternalInput")
    dout = nc.dram_tensor("out", [N, E], FP32, kind="ExternalOutput")
    drs = nc.dram_tensor("r_scratch", [H * N], FP32)

    with tile.TileContext(nc) as tc, ExitStack() as ctx:
        _body(ctx, tc, inv_tau, dQ, dK, dV, dWq, dWk, dWv, dWo, dbo, dout, drs)
    nc.finalize()
    return nc


def _body(ctx, tc, inv_tau, dQ, dK, dV, dWq, dWk, dWv, dWo, dbo, dout, drs):
    nc = tc.nc
    dma = nc.sync.dma_start

    const = ctx.enter_context(tc.tile_pool(name="const", bufs=1))
    # 12 x [128, N] bf16 slots reused across phases:
    #   big_0..3:  Q^T                     (dead after qT projections)
    #   big_4..7:  K^T, then yT            (K^T dead after kT projections)
    #   big_8..11: V^T, then oTn           (V^T dead after v projections)
    big = ctx.enter_context(tc.tile_pool(name="big", bufs=1))
    proj = ctx.enter_context(tc.tile_pool(name="proj", bufs=1))
    # one PSUM pool, 8 banks:
    #   s2_0..2 (scores f32 [128,1024])          -> 6 banks
    #     3 slots so scores(k) only waits exp(k-3): with 2 the
    #     exp->scores semaphore chain paces the whole attention phase.
    #   o2 (PV accum f32 [65,512])               -> 1 bank
    #   pp_0 (proj/outproj accum f32 [128,512])  -> 1 bank
    # Stage0/boundary units borrow the s2 slots for transposes/projections
    # (harmless there: chunk0 is DMA-gated); mid-chunk fillers use pp_0 so
    # they never serialize against the scores/exp rotation; the tail
    # rotates over everything.
    psum = ctx.enter_context(tc.tile_pool(name="psum", bufs=1, space="PSUM"))
    stage = ctx.enter_context(tc.tile_pool(name="stage", bufs=4))
    p2pool = ctx.enter_context(tc.tile_pool(name="p2pool", bufs=3))
    normp = ctx.enter_context(tc.tile_pool(name="normp", bufs=1))

    ident = const.tile([P, P], BF16, name="ident", tag="ident")
    make_identity(nc, ident)

    bo_f = const.tile([1, E], FP32, name="bo_f", tag="bo_f")
    dma(out=bo_f, in_=dbo[:])
    bo_bf = const.tile([1, E], BF16, name="bo_bf", tag="bo_bf")
    nc.vector.tensor_copy(bo_bf, bo_f)
    ones_row = const.tile([1, QC], BF16, name="ones_row", tag="ones_row")
    nc.gpsimd.memset(ones_row, 1.0)

    TAIL_TAGS = ("s2_0", "s2_1", "s2_2", "pp_0")
    aux_rr = [0]

    def aux_tag(aux):
        aux_rr[0] += 1
        if aux == "pp":
            return "pp_0"
        if aux == "tail":
            return TAIL_TAGS[aux_rr[0] % 4]
        return f"s2_{aux_rr[0] % 3}"

    def tp_tile(aux):
        return psum.tile([P, E], BF16, name="tp", tag=aux_tag(aux), bufs=1)

    def pp_tile(aux):
        return psum.tile([P, QC], FP32, name="pp", tag=aux_tag(aux), bufs=1)

    cast_rr = [0]
    CAST_ENGS = (nc.vector.tensor_copy, nc.scalar.copy)
    CAST_ENGS_NOACT = (nc.vector.tensor_copy,)

    def issue_x(dX, r, tag=None):
        """Issue the [128,E] f32 DMA for row-tile r."""
        x_f32 = stage.tile([P, E], FP32, name="x_f32",
                           tag=tag or "x_f32", bufs=1 if tag else 8)
        dma(out=x_f32, in_=dX[r * P:(r + 1) * P, :])
        return x_f32

    def cast_x(x_f32, no_act):
        x_bf = stage.tile([P, E], BF16, name="x_bf", tag="x_bf", bufs=8)
        engs = CAST_ENGS_NOACT if no_act else CAST_ENGS
        engs[cast_rr[0] % len(engs)](x_bf, x_f32)
        cast_rr[0] += 1
        return x_bf

    drain_rr = [0]
    DRAIN_ENGS = (nc.vector.tensor_copy, nc.scalar.copy)

    def drain(dst, src, no_act):
        drain_rr[0] += 1
        eng = nc.vector.tensor_copy if no_act else DRAIN_ENGS[drain_rr[0] % 2]
        eng(dst, src)

    # ---- persistent SBUF tensors ----
    wt = {}
    for wname in ("q", "k", "v", "o"):
        wt[wname] = [const.tile([P, E], BF16, name=f"w{wname}T_{c}",
                                tag=f"w{wname}T_{c}") for c in range(ET)]
    xT = {}
    slot = {"Q": 0, "K": 4, "V": 8}
    for xname in ("Q", "K", "V"):
        xT[xname] = [big.tile([P, N], BF16, name=f"{xname}T_{et}",
                              tag=f"big_{slot[xname] + et}")
                     for et in range(ET)]
    qT = [proj.tile([P, N], BF16, name=f"qT_{m}", tag=f"qT_{m}")
          for m in range(ET)]
    kT = [proj.tile([P, N], BF16, name=f"kT_{m}", tag=f"kT_{m}")
          for m in range(ET)]
    v_aug = [proj.tile([P, H * HV], BF16, name=f"vaug_{nt}",
                       tag=f"vaug_{nt}") for nt in range(NT)]
    oTn = [big.tile([P, N], BF16, name=f"oTn_{m}", tag=f"big_{8 + m}")
           for m in range(ET)]
    yT = [big.tile([P, N], BF16, name=f"yT_{m}", tag=f"big_{4 + m}")
          for m in range(ET)]

    # ---- stage-0 / boundary unit builders ----
    def w_unit(wname, dW, no_act=False, aux="s2", pre=None):
        w_bfs = [cast_x(t, no_act) for t in pre] if pre is not None else \
            [cast_x(issue_x(dW, r), no_act) for r in range(ET)]
        for c in range(ET):
            tp = tp_tile(aux)
            for r in range(ET):
                nc.tensor.transpose(
                    tp[:, r * P:(r + 1) * P], w_bfs[r][:, c * P:(c + 1) * P],
                    ident)
            drain(wt[wname][c], tp, no_act)

    def x_group(xname, dX, g, no_act=False, aux="s2", pre=None,
                pre_bf=None):
        """Cast+transpose rows [g*512, (g+1)*512) of input xname."""
        if pre_bf is not None:
            x_bfs = pre_bf
        else:
            x_f32s = pre if pre is not None else \
                [issue_x(dX, g * ET + i) for i in range(ET)]
            x_bfs = [cast_x(t, no_act) for t in x_f32s]
        for et in range(ET):
            tp = tp_tile(aux)
            for i in range(ET):
                nc.tensor.transpose(
                    tp[:, i * P:(i + 1) * P],
                    x_bfs[i][:, et * P:(et + 1) * P], ident)
            drain(xT[xname][et][:, g * E:(g + 1) * E], tp, no_act)

    def emit_qk_proj(pname, m, c, aux="s2"):
        outs = qT if pname == "q" else kT
        xtiles = xT["Q"] if pname == "q" else xT["K"]
        ps = pp_tile(aux)
        for et in range(ET):
            nc.tensor.matmul(
                ps,
                lhsT=wt[pname][et][:, m * P:(m + 1) * P],
                rhs=xtiles[et][:, c * QC:(c + 1) * QC],
                start=(et == 0), stop=(et == ET - 1))
        nc.vector.tensor_copy(outs[m][:, c * QC:(c + 1) * QC], ps)

    def emit_v_proj(nt, aux="s2"):
        ps = pp_tile(aux)
        for et in range(ET):
            nc.tensor.matmul(
                ps,
                lhsT=xT["V"][et][:, nt * P:(nt + 1) * P],
                rhs=wt["v"][et],
                start=(et == 0), stop=(et == ET - 1))
        va = v_aug[nt].rearrange("p (h c) -> p h c", c=HV)
        nc.vector.tensor_copy(
            va[:, :, 0:D], ps.rearrange("p (h d) -> p h d", d=D))
        nc.gpsimd.memset(va[:, :, D:HV], 1.0)

    # ---- attention ----
    def emit_pv(h, o2, p2, ktp):
        for j in range(2):
            kt = 2 * ktp + j
            nc.tensor.matmul(
                o2[:, :],
                lhsT=v_aug[kt][:, h * HV:(h + 1) * HV],
                rhs=p2[:, j * QC:(j + 1) * QC],
                start=(kt == 0), stop=(kt == NT - 1),
                skip_group_check=True)

    def emit_norm(h, qc, o2):
        """Normalize o2 -> oTn[hp][h2 slice, qc chunk]. No PE work.
        o2-freeing copies run now; the reciprocal rides a cheap
        partition-spread ([64,8]); the broadcast-dependent multiply is
        deferred into the next chunk so the DVE queue never stalls ahead
        of the next o2 drain."""
        hp, h2 = h // 2, (h % 2) * 64
        ot_s = normp.tile([P, QC], BF16, name="ot_s", tag="ot_s", bufs=3)
        nc.vector.tensor_copy(ot_s[h2:h2 + D, :], o2[0:D, :])
        ls = normp.tile([1, QC], FP32, name="ls", tag="ls", bufs=2)
        nc.vector.tensor_copy(ls, o2[D:HV, :])
        ltmp = normp.tile([D, QC // D], FP32, name="ltmp", tag="ltmp", bufs=2)
        dma(out=ltmp, in_=ls)
        nc.vector.reciprocal(ltmp, ltmp)
        off = h * N + qc * QC
        dma(out=drs[off:off + QC], in_=ltmp)
        rb = normp.tile([P, QC], FP32, name="rb", tag="rb", bufs=2)
        bsrc = bass.AP(tensor=drs, offset=off, ap=[[0, D], [1, QC]])
        dma(out=rb[h2:h2 + D, :], in_=bsrc)
        nc.vector.tensor_tensor(
            oTn[hp][h2:h2 + D, qc * QC:(qc + 1) * QC],
            ot_s[h2:h2 + D, :], rb[h2:h2 + D, :], ALU.mult)

    s2_rr = [0]
    pend = [None]  # previous chunk's final PV + norm, emitted after the
    #              # next chunk's first scores pair (hides the exp->PV gate)

    def attn_chunk(h, qc, weave=None):
        """weave: dict ktp -> list of callables, emitted after that
        ktp's exp."""
        hp, h2 = h // 2, (h % 2) * 64
        o2 = psum.tile([HV, QC], FP32, name="o2", tag="o2", bufs=1)
        rhs_q = qT[hp][h2:h2 + 64, qc * QC:(qc + 1) * QC]
        prev = None  # (p2, ktp) pending PV
        for ktp in range(KTP):
            s2_rr[0] += 1
            s2 = psum.tile([P, 2 * QC], FP32, name="s2",
                           tag=f"s2_{s2_rr[0] % 3}", bufs=1)
            for j in range(2):
                kt = 2 * ktp + j
                lhs_k = kT[hp][h2:h2 + 64, kt * P:(kt + 1) * P]
                nc.tensor.matmul(s2[:, j * QC:(j + 1) * QC], lhsT=lhs_k,
                                 rhs=rhs_q, start=True, stop=True)
            if ktp == 0 and pend[0]:
                pend[0]()
                pend[0] = None
            if prev is not None:
                emit_pv(h, o2, *prev)
            p2 = p2pool.tile([P, 2 * QC], BF16, name="p2", tag="p2")
            nc.scalar.activation(p2, s2, AF.Exp, scale=inv_tau)
            prev = (p2, ktp)
            if weave is not None and ktp in weave:
                for fn in weave[ktp]:
                    fn()

        def final(h=h, qc=qc, o2=o2, prev=prev):
            emit_pv(h, o2, *prev)
            emit_norm(h, qc, o2)
        pend[0] = final

    # ---- output projection + store units (fillers in late chunks) ----
    def emit_out_proj(m, c, aux="pp"):
        ps = pp_tile(aux)
        for et in range(ET):
            nc.tensor.matmul(
                ps,
                lhsT=wt["o"][et][:, m * P:(m + 1) * P],
                rhs=oTn[et][:, c * QC:(c + 1) * QC],
                start=(et == 0), stop=False)
        nc.tensor.matmul(ps, lhsT=bo_bf[0:1, m * P:(m + 1) * P],
                         rhs=ones_row, start=False, stop=True)
        nc.vector.tensor_copy(yT[m][:, c * QC:(c + 1) * QC], ps)

    store_rr = [0]

    def emit_store(nt, aux="pp"):
        tp = tp_tile(aux)
        for m in range(ET):
            nc.tensor.transpose(
                tp[:, m * P:(m + 1) * P],
                yT[m][:, nt * P:(nt + 1) * P], ident)
        y_sb = stage.tile([P, E], FP32, name="y_sb", tag="y_sb", bufs=6)
        store_rr[0] += 1
        if aux == "tail" and store_rr[0] % 2:
            nc.scalar.copy(y_sb, tp)
        else:
            nc.vector.tensor_copy(y_sb, tp)
        dma(out=dout[nt * P:(nt + 1) * P, :], in_=y_sb)

    # ================= emission =================
    # DMA issue order == emission order:
    #   Wv Wk Wq | V0..V3 | K0 Q0 | K1 K2 K3 Q1 Q2 Q3 (woven in chunk0) | Wo
    w_unit("v", dWv)
    w_unit("k", dWk)
    w_unit("q", dWq)
    for g in range(NG):
        x_group("V", dV, g)
        for nt in range(4 * g, 4 * g + 4):
            emit_v_proj(nt, aux="pp")
    x_group("K", dK, 0)
    emit_qk_proj("k", 0, 0, aux="pp")
    x_group("Q", dQ, 0)
    emit_qk_proj("q", 0, 0, aux="pp")

    q_pre = {}

    def k_unit(g):
        def fn():
            x_group("K", dK, g, no_act=True, aux="s2")
            emit_qk_proj("k", 0, g, aux="pp")
        return fn

    def q_issue(g):
        def fn():
            q_pre[g] = [issue_x(dQ, g * ET + i, tag=f"qs_{g}_{i}")
                        for i in range(ET)]
        return fn

    q_bf = {}

    def q_cast(c):
        def fn():
            q_bf[c] = [cast_x(t, True) for t in q_pre.pop(c)]
        return fn

    def q_unit(c):
        x_group("Q", dQ, c, no_act=True, aux="s2", pre_bf=q_bf.pop(c))
        emit_qk_proj("q", 0, c, aux="pp")

    # chunk (h0,qc0): weave K groups just ahead of the ktps that need
    # them, and pre-issue the Q1..Q3 DMAs so later chunks never wait.
    wo_pre = []

    def wo_issue():
        wo_pre.extend(issue_x(dWo, r, tag=f"wos_{r}") for r in range(ET))

    attn_chunk(0, 0, weave={0: [k_unit(1)], 1: [k_unit(2)], 2: [k_unit(3)],
                            3: [q_issue(1)], 4: [q_issue(2)],
                            5: [q_issue(3)], 6: [wo_issue],
                            7: [q_cast(1)]})
    q_unit(1)
    attn_chunk(0, 1, weave={5: [q_cast(2)]})
    q_unit(2)
    attn_chunk(0, 2, weave={5: [q_cast(3)]})
    q_unit(3)
    attn_chunk(0, 3)

    # filler deque for chunks h>=1: Wo prep + remaining projections,
    # then output projection / store units pinned to late chunks.
    wo_bf = []

    def wo_cast_piece():
        wo_bf.extend(cast_x(t, True) for t in wo_pre)

    def wo_piece(c):
        def fn():
            tp = tp_tile("pp")
            for r in range(ET):
                nc.tensor.transpose(
                    tp[:, r * P:(r + 1) * P], wo_bf[r][:, c * P:(c + 1) * P],
                    ident)
            drain(wt["o"][c], tp, True)
        return fn

    filler = deque()
    filler.append(lambda: (wo_cast_piece(), wo_piece(0)()))
    for m in range(1, ET):
        for c in range(NQC):
            filler.append(
                lambda m=m, c=c: emit_qk_proj("k", m, c, aux="pp"))
        for c in range(NQC):
            filler.append(
                lambda m=m, c=c: emit_qk_proj("q", m, c, aux="pp"))
    for c in range(1, ET):
        filler.append(wo_piece(c))

    def chunk_fillers(ci):
        """fillers for chunk index ci (4..31): pop deque at ktps 1,5;
        pinned outproj/store units in chunks 29..31."""
        w = {}
        if ci >= 29:
            c = ci - 29  # outproj col chunk ready during this chunk
            for s, m in zip((1, 3, 5, 7), range(ET)):
                w[s] = [lambda m=m, c=c: emit_out_proj(m, c)]
        else:
            for s in (1, 5):
                if filler:
                    w[s] = [filler.popleft()]
        return w

    for h in range(1, H):
        for qc in range(NQC):
            attn_chunk(h, qc, weave=chunk_fillers(h * NQC + qc))

    # ---- tail: last outproj column + all stores ----
    if pend[0]:
        pend[0]()
        pend[0] = None
    for m in range(ET):
        emit_out_proj(m, 3, aux="tail")
    for nt in range(NT):
        emit_store(nt, aux="tail")


_CACHE = {}


def _get_nc(inv_tau: float) -> bass.Bass:
    key = round(float(inv_tau), 9)
    if key not in _CACHE:
        _CACHE[key] = _build(float(inv_tau))
    return _CACHE[key]


def _run(inputs: dict, trace: bool = False):
    """Returns (output [B,N,E] fp32, BassKernelResults)."""
    from concourse.bass_utils import run_bass_kernel_spmd

    Q = np.ascontiguousarray(np.asarray(inputs["Q"], dtype=np.float32))
    K = np.ascontiguousarray(np.asarray(inputs["K"], dtype=np.float32))
    V = np.ascontiguousarray(np.asarray(inputs["V"], dtype=np.float32))
    Wq = np.ascontiguousarray(np.asarray(inputs["Wq"], dtype=np.float32))
    Wk = np.ascontiguousarray(np.asarray(inputs["Wk"], dtype=np.float32))
    Wv = np.ascontiguousarray(np.asarray(inputs["Wv"], dtype=np.float32))
    Wo = np.ascontiguousarray(np.asarray(inputs["Wo"], dtype=np.float32))
    bo = np.ascontiguousarray(np.asarray(inputs["bo"], dtype=np.float32))
    tau = float(np.asarray(inputs["tau"]))

    mask = inputs.get("attn_mask")
    if mask is not None and not np.all(np.asarray(mask) != 0):
        # Fallback (never hit for the spec'd all-ones mask): host math.
        return _host_reference(Q, K, V, np.asarray(mask), Wq, Wk, Wv, Wo,
                               bo, tau), None

    nc = _get_nc(1.0 / tau)
    in_maps = []
    for b in range(NCORES):
        in_maps.append({
            "Q": Q[b], "K": K[b], "V": V[b],
            "Wq": Wq, "Wk": Wk, "Wv": Wv, "Wo": Wo, "bo": bo,
        })
    res = run_bass_kernel_spmd(nc, in_maps, list(range(NCORES)), trace=trace)
    out = np.stack([np.asarray(res.results[b]["out"]) for b in range(NCORES)])
    return out.astype(np.float32), res


def _host_reference(Q, K, V, mask, Wq, Wk, Wv, Wo, bo, tau):
    b, n, _ = Q.shape
    q = (Q @ Wq.T).reshape(b, n, H, D).transpose(0, 2, 1, 3)
    k = (K @ Wk.T).reshape(b, n, H, D).transpose(0, 2, 1, 3)
    v = (V @ Wv.T).reshape(b, n, H, D).transpose(0, 2, 1, 3)
    s = np.einsum("bhnd,bhmd->bhnm", q, k) / tau
    s = np.where(mask == 0, -np.inf, s)
    s = s - s.max(axis=-1, keepdims=True)
    e = np.exp(s)
    a = e / e.sum(axis=-1, keepdims=True)
    o = np.einsum("bhnm,bhmd->bhnd", a, v)
    o = o.transpose(0, 2, 1, 3).reshape(b, n, H * D)
    return (o @ Wo.T + bo).astype(np.float32)


def kernel(**inputs) -> np.ndarray:
    out, _ = _run(inputs, trace=False)
    return out


# revision 27
# speedup vs baseline: 1.1847x; 1.1847x over previous
"""MultiHeadSelfAttention Trainium2 Bass kernel.

Shapes (hardcoded): B=8, N=2048, E=512, H=8 heads, D=64 head dim.
Sharding: data-parallel over batch -> one batch item per NeuronCore (8 cores),
no collectives needed.

Per-core pipeline (bf16 compute, fp32 accumulate). The attention phase is
bound by the Activation engine (the only engine with Exp): 256 exps of
[128,1024] at ~1.09us each. Everything else is arranged so no other engine
ever blocks that stream:

  loads: DMA order Wv,Wk,Wq | V | K0 Q0 K1..K3 | Q1..Q3 | Wo. Per-512-row
         group: f32 load, bf16 cast (DVE/ACT/GPSIMD round-robin),
         PE-transpose into X^T, project (v_aug / kT / qT) as inputs land.
         The first attention chunk is woven between K-group arrivals so the
         Exp stream starts as early as possible.
  attention (h outer, 512-query chunk inner, key-tile pair per step):
         S^T [128 keys, 2x512] in PSUM = k^T . q^T; Exp on ScalarE (1/tau
         folded into scale) -> P^T bf16; O^T [65,512] += v_aug^T . P^T over
         16 key tiles; row 64 = softmax denominator (ones column of v_aug).
         Remaining qk projections, Wo prep, output projection and stores are
         interleaved as PE filler.
  norm (per chunk, eager, no PE): O row 64 -> DRAM, broadcast-DMA back
         across 64 partitions, reciprocal_approx_fast, multiply into oTn.
  tail: yT = Wo @ oTn + bo per column chunk as h7 filler; PE-transpose back
         to [2048,512]; cast fp32; DMA out.

The attention mask is all ones per the problem spec; validated host-side.
"""

import sys

for _p in ("/opt/trn_rl_repo",):
    if _p not in sys.path:
        sys.path.insert(0, _p)

import numpy as np
from collections import deque
from contextlib import ExitStack

import concourse.bass as bass
import concourse.bacc as bacc
import concourse.mybir as mybir
import concourse.tile as tile
from concourse.masks import make_identity

B, N, E = 8, 2048, 512
H, D = 8, 64
P = 128          # partitions
ET = E // P      # 4 e-tiles
NT = N // P      # 16 n-tiles
QC = 512         # q chunk in attention
NQC = N // QC    # 4
KTP = NT // 2    # 8 key-tile pairs
NG = NT // ET    # 4 row-groups of 512 per input
HV = 65          # head dim + ones column
FP32 = mybir.dt.float32
BF16 = mybir.dt.bfloat16
NCORES = 8

AF = mybir.ActivationFunctionType
ALU = mybir.AluOpType


def _build(inv_tau: float) -> bass.Bass:
    nc = bacc.Bacc(trn_type="TRN2")

    dQ = nc.dram_tensor("Q", [N, E], FP32, kind="ExternalInput")
    dK = nc.dram_tensor("K", [N, E], FP32, kind="ExternalInput")
    dV = nc.dram_tensor("V", [N, E], FP32, kind="ExternalInput")
    dWq = nc.dram_tensor("Wq", [E, E], FP32, kind="ExternalInput")
    dWk = nc.dram_tensor("Wk", [E, E], FP32, kind="ExternalInput")
    dWv = nc.dram_tensor("Wv", [E, E], FP32, kind="ExternalInput")
    dWo = nc.dram_tensor("Wo", [E, E], FP32, kind="ExternalInput")
    dbo = nc.dram_tensor("bo", [E], FP32, kind="ExternalInput")
    dout = nc.dram_tensor("out", [N, E], FP32, kind="ExternalOutput")
    drs = nc.dram_tensor("r_scratch", [H * N], FP32)

    with tile.TileContext(nc) as tc, ExitStack() as ctx:
        _body(ctx, tc, inv_tau, dQ, dK, dV, dWq, dWk, dWv, dWo, dbo, dout, drs)
    nc.finalize()
    return nc


def _body(ctx, tc, inv_tau, dQ, dK, dV, dWq, dWk, dWv, dWo, dbo, dout, drs):
    nc = tc.nc
    dma = nc.sync.dma_start

    const = ctx.enter_context(tc.tile_pool(name="const", bufs=1))
    # 12 x [128, N] bf16 slots reused across phases:
    #   big_0..3:  Q^T                     (dead after qT projections)
    #   big_4..7:  K^T, then yT            (K^T dead after kT projections)
    #   big_8..11: V^T, then oTn           (V^T dead after v projections)
    big = ctx.enter_context(tc.tile_pool(name="big", bufs=1))
    proj = ctx.enter_context(tc.tile_pool(name="proj", bufs=1))
    # one PSUM pool, 8 banks:
    #   s2_0..2 (scores f32 [128,1024])          -> 6 banks
    #     3 slots so scores(k) only waits exp(k-3): with 2 the
    #     exp->scores semaphore chain paces the whole attention phase.
    #   o2 (PV accum f32 [65,512])               -> 1 bank
    #   pp_0 (proj/outproj accum f32 [128,512])  -> 1 bank
    # Stage0/boundary units borrow the s2 slots for transposes/projections
    # (harmless there: chunk0 is DMA-gated); mid-chunk fillers use pp_0 so
    # they never serialize against the scores/exp rotation; the tail
    # rotates over everything.
    psum = ctx.enter_context(tc.tile_pool(name="psum", bufs=1, space="PSUM"))
    stage = ctx.enter_context(tc.tile_pool(name="stage", bufs=4))
    p2pool = ctx.enter_context(tc.tile_pool(name="p2pool", bufs=3))
    normp = ctx.enter_context(tc.tile_pool(name="normp", bufs=1))

    ident = const.tile([P, P], BF16, name="ident", tag="ident")
    make_identity(nc, ident)

    bo_sb = const.tile([P, ET], FP32, name="bo_sb", tag="bo_sb")
    dma(out=bo_sb, in_=dbo[:].rearrange("(t p) -> p t", p=P))

    TAIL_TAGS = ("s2_0", "s2_1", "s2_2", "pp_0")
    aux_rr = [0]

    def aux_tag(aux):
        aux_rr[0] += 1
        if aux == "pp":
            return "pp_0"
        if aux == "tail":
            return TAIL_TAGS[aux_rr[0] % 4]
        return f"s2_{aux_rr[0] % 3}"

    def tp_tile(aux):
        return psum.tile([P, E], BF16, name="tp", tag=aux_tag(aux), bufs=1)

    def pp_tile(aux):
        return psum.tile([P, QC], FP32, name="pp", tag=aux_tag(aux), bufs=1)

    cast_rr = [0]
    CAST_ENGS = (nc.vector.tensor_copy, nc.scalar.copy)
    CAST_ENGS_NOACT = (nc.vector.tensor_copy,)

    def issue_x(dX, r, tag=None):
        """Issue the [128,E] f32 DMA for row-tile r."""
        x_f32 = stage.tile([P, E], FP32, name="x_f32",
                           tag=tag or "x_f32", bufs=1 if tag else 8)
        dma(out=x_f32, in_=dX[r * P:(r + 1) * P, :])
        return x_f32

    def cast_x(x_f32, no_act):
        x_bf = stage.tile([P, E], BF16, name="x_bf", tag="x_bf", bufs=8)
        engs = CAST_ENGS_NOACT if no_act else CAST_ENGS
        engs[cast_rr[0] % len(engs)](x_bf, x_f32)
        cast_rr[0] += 1
        return x_bf

    drain_rr = [0]
    DRAIN_ENGS = (nc.vector.tensor_copy, nc.scalar.copy)

    def drain(dst, src, no_act):
        drain_rr[0] += 1
        eng = nc.vector.tensor_copy if no_act else DRAIN_ENGS[drain_rr[0] % 2]
        eng(dst, src)

    # ---- persistent SBUF tensors ----
    wt = {}
    for wname in ("q", "k", "v", "o"):
        wt[wname] = [const.tile([P, E], BF16, name=f"w{wname}T_{c}",
                                tag=f"w{wname}T_{c}") for c in range(ET)]
    xT = {}
    slot = {"Q": 0, "K": 4, "V": 8}
    for xname in ("Q", "K", "V"):
        xT[xname] = [big.tile([P, N], BF16, name=f"{xname}T_{et}",
                              tag=f"big_{slot[xname] + et}")
                     for et in range(ET)]
    qT = [proj.tile([P, N], BF16, name=f"qT_{m}", tag=f"qT_{m}")
          for m in range(ET)]
    kT = [proj.tile([P, N], BF16, name=f"kT_{m}", tag=f"kT_{m}")
          for m in range(ET)]
    v_aug = [proj.tile([P, H * HV], BF16, name=f"vaug_{nt}",
                       tag=f"vaug_{nt}") for nt in range(NT)]
    oTn = [big.tile([P, N], BF16, name=f"oTn_{m}", tag=f"big_{8 + m}")
           for m in range(ET)]
    yT = [big.tile([P, N], BF16, name=f"yT_{m}", tag=f"big_{4 + m}")
          for m in range(ET)]

    # ---- stage-0 / boundary unit builders ----
    def w_unit(wname, dW, no_act=False, aux="s2", pre=None):
        w_bfs = [cast_x(t, no_act) for t in pre] if pre is not None else \
            [cast_x(issue_x(dW, r), no_act) for r in range(ET)]
        for c in range(ET):
            tp = tp_tile(aux)
            for r in range(ET):
                nc.tensor.transpose(
                    tp[:, r * P:(r + 1) * P], w_bfs[r][:, c * P:(c + 1) * P],
                    ident)
            drain(wt[wname][c], tp, no_act)

    def x_group(xname, dX, g, no_act=False, aux="s2", pre=None,
                pre_bf=None):
        """Cast+transpose rows [g*512, (g+1)*512) of input xname."""
        if pre_bf is not None:
            x_bfs = pre_bf
        else:
            x_f32s = pre if pre is not None else \
                [issue_x(dX, g * ET + i) for i in range(ET)]
            x_bfs = [cast_x(t, no_act) for t in x_f32s]
        for et in range(ET):
            tp = tp_tile(aux)
            for i in range(ET):
                nc.tensor.transpose(
                    tp[:, i * P:(i + 1) * P],
                    x_bfs[i][:, et * P:(et + 1) * P], ident)
            drain(xT[xname][et][:, g * E:(g + 1) * E], tp, no_act)

    def emit_qk_proj(pname, m, c, aux="s2"):
        outs = qT if pname == "q" else kT
        xtiles = xT["Q"] if pname == "q" else xT["K"]
        ps = pp_tile(aux)
        for et in range(ET):
            nc.tensor.matmul(
                ps,
                lhsT=wt[pname][et][:, m * P:(m + 1) * P],
                rhs=xtiles[et][:, c * QC:(c + 1) * QC],
                start=(et == 0), stop=(et == ET - 1))
        nc.vector.tensor_copy(outs[m][:, c * QC:(c + 1) * QC], ps)

    def emit_v_proj(nt, aux="s2"):
        ps = pp_tile(aux)
        for et in range(ET):
            nc.tensor.matmul(
                ps,
                lhsT=xT["V"][et][:, nt * P:(nt + 1) * P],
                rhs=wt["v"][et],
                start=(et == 0), stop=(et == ET - 1))
        va = v_aug[nt].rearrange("p (h c) -> p h c", c=HV)
        nc.vector.tensor_copy(
            va[:, :, 0:D], ps.rearrange("p (h d) -> p h d", d=D))
        nc.gpsimd.memset(va[:, :, D:HV], 1.0)

    # ---- attention ----
    def emit_pv(h, o2, p2, ktp):
        for j in range(2):
            kt = 2 * ktp + j
            nc.tensor.matmul(
                o2[:, :],
                lhsT=v_aug[kt][:, h * HV:(h + 1) * HV],
                rhs=p2[:, j * QC:(j + 1) * QC],
                start=(kt == 0), stop=(kt == NT - 1),
                skip_group_check=True)

    def emit_norm(h, qc, o2):
        """Normalize o2 -> oTn[hp][h2 slice, qc chunk]. No PE work.
        o2-freeing copies run now; the reciprocal rides a cheap
        partition-spread ([64,8]); the broadcast-dependent multiply is
        deferred into the next chunk so the DVE queue never stalls ahead
        of the next o2 drain."""
        hp, h2 = h // 2, (h % 2) * 64
        ot_s = normp.tile([P, QC], BF16, name="ot_s", tag="ot_s", bufs=3)
        nc.vector.tensor_copy(ot_s[h2:h2 + D, :], o2[0:D, :])
        ls = normp.tile([1, QC], FP32, name="ls", tag="ls", bufs=2)
        nc.vector.tensor_copy(ls, o2[D:HV, :])
        ltmp = normp.tile([D, QC // D], FP32, name="ltmp", tag="ltmp", bufs=2)
        dma(out=ltmp, in_=ls)
        nc.vector.reciprocal(ltmp, ltmp)
        off = h * N + qc * QC
        dma(out=drs[off:off + QC], in_=ltmp)
        rb = normp.tile([P, QC], FP32, name="rb", tag="rb", bufs=2)
        bsrc = bass.AP(tensor=drs, offset=off, ap=[[0, D], [1, QC]])
        dma(out=rb[h2:h2 + D, :], in_=bsrc)
        nc.vector.tensor_tensor(
            oTn[hp][h2:h2 + D, qc * QC:(qc + 1) * QC],
            ot_s[h2:h2 + D, :], rb[h2:h2 + D, :], ALU.mult)

    s2_rr = [0]
    pend = [None]  # previous chunk's final PV + norm, emitted after the
    #              # next chunk's first scores pair (hides the exp->PV gate)

    def attn_chunk(h, qc, weave=None):
        """weave: dict ktp -> list of callables, emitted after that
        ktp's exp."""
        hp, h2 = h // 2, (h % 2) * 64
        o2 = psum.tile([HV, QC], FP32, name="o2", tag="o2", bufs=1)
        rhs_q = qT[hp][h2:h2 + 64, qc * QC:(qc + 1) * QC]
        prev = None  # (p2, ktp) pending PV
        for ktp in range(KTP):
            s2_rr[0] += 1
            s2 = psum.tile([P, 2 * QC], FP32, name="s2",
                           tag=f"s2_{s2_rr[0] % 3}", bufs=1)
            for j in range(2):
                kt = 2 * ktp + j
                lhs_k = kT[hp][h2:h2 + 64, kt * P:(kt + 1) * P]
                nc.tensor.matmul(s2[:, j * QC:(j + 1) * QC], lhsT=lhs_k,
                                 rhs=rhs_q, start=True, stop=True)
            if ktp == 0 and pend[0]:
                pend[0]()
                pend[0] = None
            if prev is not None:
                emit_pv(h, o2, *prev)
            p2 = p2pool.tile([P, 2 * QC], BF16, name="p2", tag="p2")
            nc.scalar.activation(p2, s2, AF.Exp, scale=inv_tau)
            prev = (p2, ktp)
            if weave is not None and ktp in weave:
                for fn in weave[ktp]:
                    fn()

        def final(h=h, qc=qc, o2=o2, prev=prev):
            emit_pv(h, o2, *prev)
            emit_norm(h, qc, o2)
        pend[0] = final

    # ---- output projection + store units (fillers in late chunks) ----
    def emit_out_proj(m, c, aux="pp"):
        ps = pp_tile(aux)
        for et in range(ET):
            nc.tensor.matmul(
                ps,
                lhsT=wt["o"][et][:, m * P:(m + 1) * P],
                rhs=oTn[et][:, c * QC:(c + 1) * QC],
                start=(et == 0), stop=(et == ET - 1))
        nc.scalar.add(yT[m][:, c * QC:(c + 1) * QC], ps,
                      bo_sb[:, m:m + 1])

    store_rr = [0]

    def emit_store(nt, aux="pp"):
        tp = tp_tile(aux)
        for m in range(ET):
            nc.tensor.transpose(
                tp[:, m * P:(m + 1) * P],
                yT[m][:, nt * P:(nt + 1) * P], ident)
        y_sb = stage.tile([P, E], FP32, name="y_sb", tag="y_sb", bufs=6)
        store_rr[0] += 1
        if aux == "tail" and store_rr[0] % 2:
            nc.scalar.copy(y_sb, tp)
        else:
            nc.vector.tensor_copy(y_sb, tp)
        dma(out=dout[nt * P:(nt + 1) * P, :], in_=y_sb)

    # ================= emission =================
    # DMA issue order == emission order:
    #   Wv Wk Wq | V0..V3 | K0 Q0 | K1 K2 K3 Q1 Q2 Q3 (woven in chunk0) | Wo
    w_unit("v", dWv)
    w_unit("k", dWk)
    w_unit("q", dWq)
    for g in range(NG):
        x_group("V", dV, g)
        for nt in range(4 * g, 4 * g + 4):
            emit_v_proj(nt, aux="pp")
    x_group("K", dK, 0)
    emit_qk_proj("k", 0, 0, aux="pp")
    x_group("Q", dQ, 0)
    emit_qk_proj("q", 0, 0, aux="pp")

    q_pre = {}

    def k_unit(g):
        def fn():
            x_group("K", dK, g, no_act=True, aux="s2")
            emit_qk_proj("k", 0, g, aux="pp")
        return fn

    def q_issue(g):
        def fn():
            q_pre[g] = [issue_x(dQ, g * ET + i, tag=f"qs_{g}_{i}")
                        for i in range(ET)]
        return fn

    q_bf = {}

    def q_cast(c):
        def fn():
            q_bf[c] = [cast_x(t, True) for t in q_pre.pop(c)]
        return fn

    def q_unit(c):
        x_group("Q", dQ, c, no_act=True, aux="s2", pre_bf=q_bf.pop(c))
        emit_qk_proj("q", 0, c, aux="pp")

    # chunk (h0,qc0): weave K groups just ahead of the ktps that need
    # them, and pre-issue the Q1..Q3 DMAs so later chunks never wait.
    wo_pre = []

    def wo_issue():
        wo_pre.extend(issue_x(dWo, r, tag=f"wos_{r}") for r in range(ET))

    attn_chunk(0, 0, weave={0: [k_unit(1)], 1: [k_unit(2)], 2: [k_unit(3)],
                            3: [q_issue(1)], 4: [q_issue(2)],
                            5: [q_issue(3)], 6: [wo_issue],
                            7: [q_cast(1)]})
    q_unit(1)
    attn_chunk(0, 1, weave={5: [q_cast(2)]})
    q_unit(2)
    attn_chunk(0, 2, weave={5: [q_cast(3)]})
    q_unit(3)
    attn_chunk(0, 3)

    # filler deque for chunks h>=1: Wo prep + remaining projections,
    # then output projection / store units pinned to late chunks.
    wo_bf = []

    def wo_cast_piece():
        wo_bf.extend(cast_x(t, True) for t in wo_pre)

    def wo_piece(c):
        def fn():
            tp = tp_tile("pp")
            for r in range(ET):
                nc.tensor.transpose(
                    tp[:, r * P:(r + 1) * P], wo_bf[r][:, c * P:(c + 1) * P],
                    ident)
            drain(wt["o"][c], tp, True)
        return fn

    filler = deque()
    filler.append(lambda: (wo_cast_piece(), wo_piece(0)()))
    for m in range(1, ET):
        for c in range(NQC):
            filler.append(
                lambda m=m, c=c: emit_qk_proj("k", m, c, aux="pp"))
        for c in range(NQC):
            filler.append(
                lambda m=m, c=c: emit_qk_proj("q", m, c, aux="pp"))
    for c in range(1, ET):
        filler.append(wo_piece(c))

    def chunk_fillers(ci):
        """fillers for chunk index ci (4..31): pop deque at ktps 1,5;
        pinned outproj/store units in chunks 29..31."""
        w = {}
        if ci >= 29:
            c = ci - 29  # outproj col chunk ready during this chunk
            for s, m in zip((1, 3, 5, 7), range(ET)):
                w[s] = [lambda m=m, c=c: emit_out_proj(m, c)]
        else:
            for s in (1, 5):
                if filler:
                    w[s] = [filler.popleft()]
        return w

    for h in range(1, H):
        for qc in range(NQC):
            attn_chunk(h, qc, weave=chunk_fillers(h * NQC + qc))

    # ---- tail: last outproj column + all stores ----
    if pend[0]:
        pend[0]()
        pend[0] = None
    for m in range(ET):
        emit_out_proj(m, 3, aux="tail")
    for nt in range(NT):
        emit_store(nt, aux="tail")


_CACHE = {}


def _get_nc(inv_tau: float) -> bass.Bass:
    key = round(float(inv_tau), 9)
    if key not in _CACHE:
        _CACHE[key] = _build(float(inv_tau))
    return _CACHE[key]


def _run(inputs: dict, trace: bool = False):
    """Returns (output [B,N,E] fp32, BassKernelResults)."""
    from concourse.bass_utils import run_bass_kernel_spmd

    Q = np.ascontiguousarray(np.asarray(inputs["Q"], dtype=np.float32))
    K = np.ascontiguousarray(np.asarray(inputs["K"], dtype=np.float32))
    V = np.ascontiguousarray(np.asarray(inputs["V"], dtype=np.float32))
    Wq = np.ascontiguousarray(np.asarray(inputs["Wq"], dtype=np.float32))
    Wk = np.ascontiguousarray(np.asarray(inputs["Wk"], dtype=np.float32))
    Wv = np.ascontiguousarray(np.asarray(inputs["Wv"], dtype=np.float32))
    Wo = np.ascontiguousarray(np.asarray(inputs["Wo"], dtype=np.float32))
    bo = np.ascontiguousarray(np.asarray(inputs["bo"], dtype=np.float32))
    tau = float(np.asarray(inputs["tau"]))

    mask = inputs.get("attn_mask")
    if mask is not None and not np.all(np.asarray(mask) != 0):
        # Fallback (never hit for the spec'd all-ones mask): host math.
        return _host_reference(Q, K, V, np.asarray(mask), Wq, Wk, Wv, Wo,
                               bo, tau), None

    nc = _get_nc(1.0 / tau)
    in_maps = []
    for b in range(NCORES):
        in_maps.append({
            "Q": Q[b], "K": K[b], "V": V[b],
            "Wq": Wq, "Wk": Wk, "Wv": Wv, "Wo": Wo, "bo": bo,
        })
    res = run_bass_kernel_spmd(nc, in_maps, list(range(NCORES)), trace=trace)
    out = np.stack([np.asarray(res.results[b]["out"]) for b in range(NCORES)])
    return out.astype(np.float32), res


def _host_reference(Q, K, V, mask, Wq, Wk, Wv, Wo, bo, tau):
    b, n, _ = Q.shape
    q = (Q @ Wq.T).reshape(b, n, H, D).transpose(0, 2, 1, 3)
    k = (K @ Wk.T).reshape(b, n, H, D).transpose(0, 2, 1, 3)
    v = (V @ Wv.T).reshape(b, n, H, D).transpose(0, 2, 1, 3)
    s = np.einsum("bhnd,bhmd->bhnm", q, k) / tau
    s = np.where(mask == 0, -np.inf, s)
    s = s - s.max(axis=-1, keepdims=True)
    e = np.exp(s)
    a = e / e.sum(axis=-1, keepdims=True)
    o = np.einsum("bhnm,bhmd->bhnd", a, v)
    o = o.transpose(0, 2, 1, 3).reshape(b, n, H * D)
    return (o @ Wo.T + bo).astype(np.float32)


def kernel(**inputs) -> np.ndarray:
    out, _ = _run(inputs, trace=False)
    return out


# revision 28
# speedup vs baseline: 1.2389x; 1.0457x over previous
"""MultiHeadSelfAttention Trainium2 Bass kernel.

Shapes (hardcoded): B=8, N=2048, E=512, H=8 heads, D=64 head dim.
Sharding: data-parallel over batch -> one batch item per NeuronCore (8 cores),
no collectives needed.

Per-core pipeline (bf16 compute, fp32 accumulate). The attention phase is
bound by the Activation engine (the only engine with Exp): 256 exps of
[128,1024] at ~1.09us each. Everything else is arranged so no other engine
ever blocks that stream:

  loads: DMA order Wv,Wk,Wq | V | K0 Q0 K1..K3 | Q1..Q3 | Wo. Per-512-row
         group: f32 load, bf16 cast (DVE/ACT/GPSIMD round-robin),
         PE-transpose into X^T, project (v_aug / kT / qT) as inputs land.
         The first attention chunk is woven between K-group arrivals so the
         Exp stream starts as early as possible.
  attention (h outer, 512-query chunk inner, key-tile pair per step):
         S^T [128 keys, 2x512] in PSUM = k^T . q^T; Exp on ScalarE (1/tau
         folded into scale) -> P^T bf16; O^T [65,512] += v_aug^T . P^T over
         16 key tiles; row 64 = softmax denominator (ones column of v_aug).
         Remaining qk projections, Wo prep, output projection and stores are
         interleaved as PE filler.
  norm (per chunk, eager, no PE): O row 64 -> DRAM, broadcast-DMA back
         across 64 partitions, reciprocal_approx_fast, multiply into oTn.
  tail: yT = Wo @ oTn + bo per column chunk as h7 filler; PE-transpose back
         to [2048,512]; cast fp32; DMA out.

The attention mask is all ones per the problem spec; validated host-side.
"""

import sys

for _p in ("/opt/trn_rl_repo",):
    if _p not in sys.path:
        sys.path.insert(0, _p)

import numpy as np
from collections import deque
from contextlib import ExitStack

import concourse.bass as bass
import concourse.bacc as bacc
import concourse.mybir as mybir
import concourse.tile as tile
from concourse.masks import make_identity

B, N, E = 8, 2048, 512
H, D = 8, 64
P = 128          # partitions
ET = E // P      # 4 e-tiles
NT = N // P      # 16 n-tiles
QC = 512         # q chunk in attention
NQC = N // QC    # 4
KTP = NT // 2    # 8 key-tile pairs
NG = NT // ET    # 4 row-groups of 512 per input
HV = 65          # head dim + ones column
FP32 = mybir.dt.float32
BF16 = mybir.dt.bfloat16
NCORES = 8

AF = mybir.ActivationFunctionType
ALU = mybir.AluOpType


def _build(inv_tau: float) -> bass.Bass:
    nc = bacc.Bacc(trn_type="TRN2")

    dQ = nc.dram_tensor("Q", [N, E], FP32, kind="ExternalInput")
    dK = nc.dram_tensor("K", [N, E], FP32, kind="ExternalInput")
    dV = nc.dram_tensor("V", [N, E], FP32, kind="ExternalInput")
    dWq = nc.dram_tensor("Wq", [E, E], FP32, kind="ExternalInput")
    dWk = nc.dram_tensor("Wk", [E, E], FP32, kind="ExternalInput")
    dWv = nc.dram_tensor("Wv", [E, E], FP32, kind="ExternalInput")
    dWo = nc.dram_tensor("Wo", [E, E], FP32, kind="ExternalInput")
    dbo = nc.dram_tensor("bo", [E], FP32, kind="ExternalInput")
    dout = nc.dram_tensor("out", [N, E], FP32, kind="ExternalOutput")
    drs = nc.dram_tensor("r_scratch", [H * N], FP32)

    with tile.TileContext(nc) as tc, ExitStack() as ctx:
        _body(ctx, tc, inv_tau, dQ, dK, dV, dWq, dWk, dWv, dWo, dbo, dout, drs)
    nc.finalize()
    return nc


def _body(ctx, tc, inv_tau, dQ, dK, dV, dWq, dWk, dWv, dWo, dbo, dout, drs):
    nc = tc.nc
    dma = nc.sync.dma_start

    const = ctx.enter_context(tc.tile_pool(name="const", bufs=1))
    # 12 x [128, N] bf16 slots reused across phases:
    #   big_0..3:  Q^T                     (dead after qT projections)
    #   big_4..7:  K^T, then yT            (K^T dead after kT projections)
    #   big_8..11: V^T, then oTn           (V^T dead after v projections)
    big = ctx.enter_context(tc.tile_pool(name="big", bufs=1))
    proj = ctx.enter_context(tc.tile_pool(name="proj", bufs=1))
    # one PSUM pool, 8 banks:
    #   s2_0..2 (scores f32 [128,1024])          -> 6 banks
    #     3 slots so scores(k) only waits exp(k-3): with 2 the
    #     exp->scores semaphore chain paces the whole attention phase.
    #   o2 (PV accum f32 [65,512])               -> 1 bank
    #   pp_0 (proj/outproj accum f32 [128,512])  -> 1 bank
    # Stage0/boundary units borrow the s2 slots for transposes/projections
    # (harmless there: chunk0 is DMA-gated); mid-chunk fillers use pp_0 so
    # they never serialize against the scores/exp rotation; the tail
    # rotates over everything.
    psum = ctx.enter_context(tc.tile_pool(name="psum", bufs=1, space="PSUM"))
    stage = ctx.enter_context(tc.tile_pool(name="stage", bufs=4))
    p2pool = ctx.enter_context(tc.tile_pool(name="p2pool", bufs=3))
    normp = ctx.enter_context(tc.tile_pool(name="normp", bufs=1))

    ident = const.tile([P, P], BF16, name="ident", tag="ident")
    make_identity(nc, ident)

    bo_sb = const.tile([P, ET], FP32, name="bo_sb", tag="bo_sb")
    dma(out=bo_sb, in_=dbo[:].rearrange("(t p) -> p t", p=P))

    TAIL_TAGS = ("s2_0", "s2_1", "s2_2", "pp_0")
    aux_rr = [0]

    def aux_tag(aux):
        aux_rr[0] += 1
        if aux == "pp":
            return "pp_0"
        if aux == "tail":
            return TAIL_TAGS[aux_rr[0] % 4]
        return f"s2_{aux_rr[0] % 3}"

    def tp_tile(aux):
        return psum.tile([P, E], BF16, name="tp", tag=aux_tag(aux), bufs=1)

    def pp_tile(aux):
        return psum.tile([P, QC], FP32, name="pp", tag=aux_tag(aux), bufs=1)

    cast_rr = [0]
    CAST_ENGS = (nc.vector.tensor_copy, nc.scalar.copy)
    CAST_ENGS_NOACT = (nc.vector.tensor_copy,)

    def issue_x(dX, r, tag=None):
        """Issue the [128,E] f32 DMA for row-tile r."""
        x_f32 = stage.tile([P, E], FP32, name="x_f32",
                           tag=tag or "x_f32", bufs=1 if tag else 8)
        dma(out=x_f32, in_=dX[r * P:(r + 1) * P, :])
        return x_f32

    def cast_x(x_f32, no_act):
        x_bf = stage.tile([P, E], BF16, name="x_bf", tag="x_bf", bufs=8)
        engs = CAST_ENGS_NOACT if no_act else CAST_ENGS
        engs[cast_rr[0] % len(engs)](x_bf, x_f32)
        cast_rr[0] += 1
        return x_bf

    drain_rr = [0]
    DRAIN_ENGS = (nc.vector.tensor_copy, nc.scalar.copy)

    def drain(dst, src, no_act):
        drain_rr[0] += 1
        eng = nc.vector.tensor_copy if no_act else DRAIN_ENGS[drain_rr[0] % 2]
        eng(dst, src)

    # ---- persistent SBUF tensors ----
    wt = {}
    for wname in ("q", "k", "v", "o"):
        wt[wname] = [const.tile([P, E], BF16, name=f"w{wname}T_{c}",
                                tag=f"w{wname}T_{c}") for c in range(ET)]
    xT = {}
    slot = {"Q": 0, "K": 4, "V": 8}
    for xname in ("Q", "K", "V"):
        xT[xname] = [big.tile([P, N], BF16, name=f"{xname}T_{et}",
                              tag=f"big_{slot[xname] + et}")
                     for et in range(ET)]
    qT = [proj.tile([P, N], BF16, name=f"qT_{m}", tag=f"qT_{m}")
          for m in range(ET)]
    kT = [proj.tile([P, N], BF16, name=f"kT_{m}", tag=f"kT_{m}")
          for m in range(ET)]
    v_aug = [proj.tile([P, H * HV], BF16, name=f"vaug_{nt}",
                       tag=f"vaug_{nt}") for nt in range(NT)]
    oTn = [big.tile([P, N], BF16, name=f"oTn_{m}", tag=f"big_{8 + m}")
           for m in range(ET)]
    yT = [big.tile([P, N], BF16, name=f"yT_{m}", tag=f"big_{4 + m}")
          for m in range(ET)]

    # ---- stage-0 / boundary unit builders ----
    def w_unit(wname, dW, no_act=False, aux="s2", pre=None):
        w_bfs = [cast_x(t, no_act) for t in pre] if pre is not None else \
            [cast_x(issue_x(dW, r), no_act) for r in range(ET)]
        for c in range(ET):
            tp = tp_tile(aux)
            for r in range(ET):
                nc.tensor.transpose(
                    tp[:, r * P:(r + 1) * P], w_bfs[r][:, c * P:(c + 1) * P],
                    ident)
            drain(wt[wname][c], tp, no_act)

    def x_group(xname, dX, g, no_act=False, aux="s2", pre=None,
                pre_bf=None):
        """Cast+transpose rows [g*512, (g+1)*512) of input xname."""
        if pre_bf is not None:
            x_bfs = pre_bf
        else:
            x_f32s = pre if pre is not None else \
                [issue_x(dX, g * ET + i) for i in range(ET)]
            x_bfs = [cast_x(t, no_act) for t in x_f32s]
        for et in range(ET):
            tp = tp_tile(aux)
            for i in range(ET):
                nc.tensor.transpose(
                    tp[:, i * P:(i + 1) * P],
                    x_bfs[i][:, et * P:(et + 1) * P], ident)
            drain(xT[xname][et][:, g * E:(g + 1) * E], tp, no_act)

    def emit_qk_proj(pname, m, c, aux="s2"):
        outs = qT if pname == "q" else kT
        xtiles = xT["Q"] if pname == "q" else xT["K"]
        ps = pp_tile(aux)
        for et in range(ET):
            nc.tensor.matmul(
                ps,
                lhsT=wt[pname][et][:, m * P:(m + 1) * P],
                rhs=xtiles[et][:, c * QC:(c + 1) * QC],
                start=(et == 0), stop=(et == ET - 1))
        nc.vector.tensor_copy(outs[m][:, c * QC:(c + 1) * QC], ps)

    def emit_v_proj(nt, aux="s2"):
        ps = pp_tile(aux)
        for et in range(ET):
            nc.tensor.matmul(
                ps,
                lhsT=xT["V"][et][:, nt * P:(nt + 1) * P],
                rhs=wt["v"][et],
                start=(et == 0), stop=(et == ET - 1))
        va = v_aug[nt].rearrange("p (h c) -> p h c", c=HV)
        nc.vector.tensor_copy(
            va[:, :, 0:D], ps.rearrange("p (h d) -> p h d", d=D))
        nc.gpsimd.memset(va[:, :, D:HV], 1.0)

    # ---- attention ----
    def emit_pv(h, o2, p2, ktp):
        for j in range(2):
            kt = 2 * ktp + j
            nc.tensor.matmul(
                o2[:, :],
                lhsT=v_aug[kt][:, h * HV:(h + 1) * HV],
                rhs=p2[:, j * QC:(j + 1) * QC],
                start=(kt == 0), stop=(kt == NT - 1),
                skip_group_check=True)

    def emit_norm(h, qc, o2):
        """Normalize o2 -> oTn[hp][h2 slice, qc chunk]. No PE work.
        o2-freeing copies run now; the reciprocal rides a cheap
        partition-spread ([64,8]); the broadcast-dependent multiply is
        deferred into the next chunk so the DVE queue never stalls ahead
        of the next o2 drain."""
        hp, h2 = h // 2, (h % 2) * 64
        ot_s = normp.tile([P, QC], BF16, name="ot_s", tag="ot_s", bufs=3)
        nc.vector.tensor_copy(ot_s[h2:h2 + D, :], o2[0:D, :])
        ls = normp.tile([1, QC], FP32, name="ls", tag="ls", bufs=2)
        nc.vector.tensor_copy(ls, o2[D:HV, :])
        ltmp = normp.tile([D, QC // D], FP32, name="ltmp", tag="ltmp", bufs=2)
        dma(out=ltmp, in_=ls)
        nc.vector.reciprocal(ltmp, ltmp)
        off = h * N + qc * QC
        dma(out=drs[off:off + QC], in_=ltmp)
        rb = normp.tile([P, QC], FP32, name="rb", tag="rb", bufs=2)
        bsrc = bass.AP(tensor=drs, offset=off, ap=[[0, D], [1, QC]])
        dma(out=rb[h2:h2 + D, :], in_=bsrc)
        nc.vector.tensor_tensor(
            oTn[hp][h2:h2 + D, qc * QC:(qc + 1) * QC],
            ot_s[h2:h2 + D, :], rb[h2:h2 + D, :], ALU.mult)

    s2_rr = [0]
    pend = [None]  # previous chunk's final PV + norm, emitted after the
    #              # next chunk's first scores pair (hides the exp->PV gate)

    def attn_chunk(h, qc, weave=None):
        """weave: dict ktp -> list of callables, emitted after that
        ktp's exp."""
        hp, h2 = h // 2, (h % 2) * 64
        o2 = psum.tile([HV, QC], FP32, name="o2", tag="o2", bufs=1)
        rhs_q = qT[hp][h2:h2 + 64, qc * QC:(qc + 1) * QC]
        prev = None  # (p2, ktp) pending PV
        for ktp in range(KTP):
            s2_rr[0] += 1
            s2 = psum.tile([P, 2 * QC], FP32, name="s2",
                           tag=f"s2_{s2_rr[0] % 3}", bufs=1)
            for j in range(2):
                kt = 2 * ktp + j
                lhs_k = kT[hp][h2:h2 + 64, kt * P:(kt + 1) * P]
                nc.tensor.matmul(s2[:, j * QC:(j + 1) * QC], lhsT=lhs_k,
                                 rhs=rhs_q, start=True, stop=True)
            if ktp == 0 and pend[0]:
                pend[0]()
                pend[0] = None
            if prev is not None:
                emit_pv(h, o2, *prev)
            p2 = p2pool.tile([P, 2 * QC], BF16, name="p2", tag="p2",
                             bufs=4)
            nc.scalar.activation(p2, s2, AF.Exp, scale=inv_tau)
            prev = (p2, ktp)
            if weave is not None and ktp in weave:
                for fn in weave[ktp]:
                    fn()

        def final(h=h, qc=qc, o2=o2, prev=prev):
            emit_pv(h, o2, *prev)
            emit_norm(h, qc, o2)
        pend[0] = final

    # ---- output projection + store units (fillers in late chunks) ----
    def emit_out_proj(m, c, aux="pp"):
        ps = pp_tile(aux)
        for et in range(ET):
            nc.tensor.matmul(
                ps,
                lhsT=wt["o"][et][:, m * P:(m + 1) * P],
                rhs=oTn[et][:, c * QC:(c + 1) * QC],
                start=(et == 0), stop=(et == ET - 1))
        nc.scalar.add(yT[m][:, c * QC:(c + 1) * QC], ps,
                      bo_sb[:, m:m + 1])

    store_rr = [0]

    def emit_store(nt, aux="pp"):
        tp = tp_tile(aux)
        for m in range(ET):
            nc.tensor.transpose(
                tp[:, m * P:(m + 1) * P],
                yT[m][:, nt * P:(nt + 1) * P], ident)
        y_sb = stage.tile([P, E], FP32, name="y_sb", tag="y_sb", bufs=6)
        store_rr[0] += 1
        if aux == "tail" and store_rr[0] % 2:
            nc.scalar.copy(y_sb, tp)
        else:
            nc.vector.tensor_copy(y_sb, tp)
        dma(out=dout[nt * P:(nt + 1) * P, :], in_=y_sb)

    # ================= emission =================
    # DMA issue order == emission order:
    #   Wv Wk Wq | V0..V3 | K0 Q0 | K1 K2 K3 Q1 Q2 Q3 (woven in chunk0) | Wo
    w_unit("v", dWv)
    w_unit("k", dWk)
    w_unit("q", dWq)
    for g in range(NG):
        x_group("V", dV, g)
        for nt in range(4 * g, 4 * g + 4):
            emit_v_proj(nt, aux="pp")
    x_group("K", dK, 0)
    emit_qk_proj("k", 0, 0, aux="pp")
    x_group("Q", dQ, 0)
    emit_qk_proj("q", 0, 0, aux="pp")

    q_pre = {}

    def k_unit(g):
        def fn():
            x_group("K", dK, g, no_act=True, aux="s2")
            emit_qk_proj("k", 0, g, aux="pp")
        return fn

    def q_issue(g):
        def fn():
            q_pre[g] = [issue_x(dQ, g * ET + i, tag=f"qs_{g}_{i}")
                        for i in range(ET)]
        return fn

    q_bf = {}

    def q_cast(c):
        def fn():
            q_bf[c] = [cast_x(t, True) for t in q_pre.pop(c)]
        return fn

    def q_unit(c):
        x_group("Q", dQ, c, no_act=True, aux="s2", pre_bf=q_bf.pop(c))
        emit_qk_proj("q", 0, c, aux="pp")

    # chunk (h0,qc0): weave K groups just ahead of the ktps that need
    # them, and pre-issue the Q1..Q3 DMAs so later chunks never wait.
    wo_pre = []

    def wo_issue():
        wo_pre.extend(issue_x(dWo, r, tag=f"wos_{r}") for r in range(ET))

    attn_chunk(0, 0, weave={0: [k_unit(1)], 1: [k_unit(2)], 2: [k_unit(3)],
                            3: [q_issue(1)], 4: [q_issue(2)],
                            5: [q_issue(3)], 6: [wo_issue],
                            7: [q_cast(1)]})
    q_unit(1)
    attn_chunk(0, 1, weave={5: [q_cast(2)]})
    q_unit(2)
    attn_chunk(0, 2, weave={5: [q_cast(3)]})
    q_unit(3)
    attn_chunk(0, 3)

    # filler deque for chunks h>=1: Wo prep + remaining projections,
    # then output projection / store units pinned to late chunks.
    wo_bf = []

    def wo_cast_piece():
        wo_bf.extend(cast_x(t, True) for t in wo_pre)

    def wo_piece(c):
        def fn():
            tp = tp_tile("pp")
            for r in range(ET):
                nc.tensor.transpose(
                    tp[:, r * P:(r + 1) * P], wo_bf[r][:, c * P:(c + 1) * P],
                    ident)
            drain(wt["o"][c], tp, True)
        return fn

    filler = deque()
    filler.append(lambda: (wo_cast_piece(), wo_piece(0)()))
    for m in range(1, ET):
        for c in range(NQC):
            filler.append(
                lambda m=m, c=c: emit_qk_proj("k", m, c, aux="pp"))
        for c in range(NQC):
            filler.append(
                lambda m=m, c=c: emit_qk_proj("q", m, c, aux="pp"))
    for c in range(1, ET):
        filler.append(wo_piece(c))

    def chunk_fillers(ci):
        """fillers for chunk index ci (4..31): pop deque at ktps 1,5;
        pinned outproj/store units in chunks 29..31."""
        w = {}
        if ci >= 29:
            c = ci - 29  # outproj col chunk ready during this chunk
            for s, m in zip((1, 3, 5, 7), range(ET)):
                w[s] = [lambda m=m, c=c: emit_out_proj(m, c)]
        else:
            for s in (1, 5):
                if filler:
                    w[s] = [filler.popleft()]
        return w

    for h in range(1, H):
        for qc in range(NQC):
            attn_chunk(h, qc, weave=chunk_fillers(h * NQC + qc))

    # ---- tail: the last chunk's norm chain (DMA round trips) hides
    # under the 12 stores that don't depend on it ----
    if pend[0]:
        pend[0]()
        pend[0] = None
    for nt in range(12):
        emit_store(nt, aux="tail")
    for m in range(ET):
        emit_out_proj(m, 3, aux="tail")
    for nt in range(12, NT):
        emit_store(nt, aux="tail")


_CACHE = {}


def _get_nc(inv_tau: float) -> bass.Bass:
    key = round(float(inv_tau), 9)
    if key not in _CACHE:
        _CACHE[key] = _build(float(inv_tau))
    return _CACHE[key]


def _run(inputs: dict, trace: bool = False):
    """Returns (output [B,N,E] fp32, BassKernelResults)."""
    from concourse.bass_utils import run_bass_kernel_spmd

    Q = np.ascontiguousarray(np.asarray(inputs["Q"], dtype=np.float32))
    K = np.ascontiguousarray(np.asarray(inputs["K"], dtype=np.float32))
    V = np.ascontiguousarray(np.asarray(inputs["V"], dtype=np.float32))
    Wq = np.ascontiguousarray(np.asarray(inputs["Wq"], dtype=np.float32))
    Wk = np.ascontiguousarray(np.asarray(inputs["Wk"], dtype=np.float32))
    Wv = np.ascontiguousarray(np.asarray(inputs["Wv"], dtype=np.float32))
    Wo = np.ascontiguousarray(np.asarray(inputs["Wo"], dtype=np.float32))
    bo = np.ascontiguousarray(np.asarray(inputs["bo"], dtype=np.float32))
    tau = float(np.asarray(inputs["tau"]))

    mask = inputs.get("attn_mask")
    if mask is not None and not np.all(np.asarray(mask) != 0):
        # Fallback (never hit for the spec'd all-ones mask): host math.
        return _host_reference(Q, K, V, np.asarray(mask), Wq, Wk, Wv, Wo,
                               bo, tau), None

    nc = _get_nc(1.0 / tau)
    in_maps = []
    for b in range(NCORES):
        in_maps.append({
            "Q": Q[b], "K": K[b], "V": V[b],
            "Wq": Wq, "Wk": Wk, "Wv": Wv, "Wo": Wo, "bo": bo,
        })
    res = run_bass_kernel_spmd(nc, in_maps, list(range(NCORES)), trace=trace)
    out = np.stack([np.asarray(res.results[b]["out"]) for b in range(NCORES)])
    return out.astype(np.float32), res


def _host_reference(Q, K, V, mask, Wq, Wk, Wv, Wo, bo, tau):
    b, n, _ = Q.shape
    q = (Q @ Wq.T).reshape(b, n, H, D).transpose(0, 2, 1, 3)
    k = (K @ Wk.T).reshape(b, n, H, D).transpose(0, 2, 1, 3)
    v = (V @ Wv.T).reshape(b, n, H, D).transpose(0, 2, 1, 3)
    s = np.einsum("bhnd,bhmd->bhnm", q, k) / tau
    s = np.where(mask == 0, -np.inf, s)
    s = s - s.max(axis=-1, keepdims=True)
    e = np.exp(s)
    a = e / e.sum(axis=-1, keepdims=True)
    o = np.einsum("bhnm,bhmd->bhnd", a, v)
    o = o.transpose(0, 2, 1, 3).reshape(b, n, H * D)
    return (o @ Wo.T + bo).astype(np.float32)


def kernel(**inputs) -> np.ndarray:
    out, _ = _run(inputs, trace=False)
    return out
